# revision 1
# baseline (speedup 1.0000x reference)
"""Trainium2 Bass kernel for nn_BlockAttnResTransformerBlock.

Computation (see reference): two sequential "inter-block attention" sub-layers.
Per token t (B*T = 8192 tokens total, all independent):
  dot_n   = <qw_phi, V_n[t]>            (qw_phi = query * res_norm_w, folded on host)
  rms_n   = rsqrt(sum(V_n[t]^2)/D + eps)
  logits  = dot_n * rms_n / sqrt(D)
  alpha   = softmax over n (9 blocks: 8 completed + partial)
  h       = sum_n alpha_n * V_n[t]
  out     = partial[t] + rmsnorm(h) @ W_eff.T      (W_eff = W * norm_w, folded)
phase 2 repeats with the updated partial and the mlp query/weights.

Sharding: data-parallel over tokens, 1024 tokens/core across 8 cores; weights
replicated. V is cast to bf16 on host in two layouts (natural [t,d] for the
PE h-build stream and d-major [d,t] chunks for the PE dot matmuls). The
residual path (partial block) stays fp32 end to end.
"""

import numpy as np
import ml_dtypes
from contextlib import ExitStack

import concourse.bass as bass
import concourse.bacc as bacc
import concourse.tile as tile
from concourse import mybir
from concourse.bass_utils import run_bass_kernel_spmd
from concourse.masks import make_identity

bf16 = ml_dtypes.bfloat16

N_BLK = 8          # completed blocks
B, T, D = 4, 2048, 1024
NCORES = 8
TOK = B * T                  # 8192
TPC = TOK // NCORES          # 1024 tokens per core
NT = TPC // 128              # 8 token-tiles per core
NCH = D // 128               # 8 d-chunks
EPS = 1e-6
INV_SCALE = 1.0 / 32.0       # 1/sqrt(D)

_BF = mybir.dt.bfloat16
_F32 = mybir.dt.float32

_CACHE = {}


def build_nc():
    nc = bacc.Bacc("TRN2", target_bir_lowering=False, debug=False)

    vn = nc.dram_tensor("vn", [NT, 128, N_BLK, D], _BF, kind="ExternalInput")
    vt = nc.dram_tensor("vt", [NT, 128, N_BLK, NCH, 128], _BF, kind="ExternalInput")
    pf = nc.dram_tensor("pf", [NT, 128, D], _F32, kind="ExternalInput")
    pb = nc.dram_tensor("pb", [NT, 128, D], _BF, kind="ExternalInput")
    qp = nc.dram_tensor("qp", [128, NCH, 2], _BF, kind="ExternalInput")
    qm = nc.dram_tensor("qm", [D], _BF, kind="ExternalInput")
    wa = nc.dram_tensor("wa", [128, NCH, D], _BF, kind="ExternalInput")
    wm = nc.dram_tensor("wm", [128, NCH, D], _BF, kind="ExternalInput")
    out = nc.dram_tensor("out", [NT, 128, D], _F32, kind="ExternalOutput")

    AF = mybir.ActivationFunctionType
    AX = mybir.AxisListType
    OP = mybir.AluOpType

    with tile.TileContext(nc) as tc, ExitStack() as ctx:
        consts = ctx.enter_context(tc.tile_pool(name="consts", bufs=1))
        vin = ctx.enter_context(tc.tile_pool(name="vin", bufs=2))
        stats = ctx.enter_context(tc.tile_pool(name="stats", bufs=6))
        work = ctx.enter_context(tc.tile_pool(name="work", bufs=3))
        pbig = ctx.enter_context(tc.tile_pool(name="pbig", bufs=3, space="PSUM"))
        pdot = ctx.enter_context(tc.tile_pool(name="pdot", bufs=2, space="PSUM"))

        ident = consts.tile([128, 128], _BF)
        make_identity(nc, ident)
        eps_sb = consts.tile([128, 1], _F32)
        nc.vector.memset(eps_sb, EPS)
        qp_sb = consts.tile([128, NCH, 2], _BF)
        nc.sync.dma_start(out=qp_sb, in_=qp[:, :, :])
        qm_bc = consts.tile([128, D], _BF)
        qm_ap = qm[:]
        nc.sync.dma_start(out=qm_bc, in_=bass.AP(
            tensor=qm_ap.tensor, offset=qm_ap.offset, ap=[[0, 128]] + list(qm_ap.ap)))
        wa_sb = consts.tile([128, NCH, D], _BF)
        nc.sync.dma_start(out=wa_sb, in_=wa[:, :, :])
        wm_sb = consts.tile([128, NCH, D], _BF)
        nc.sync.dma_start(out=wm_sb, in_=wm[:, :, :])


        _I32 = mybir.dt.int32

        def rsqrt_dve(src_ap, w, tagp):
            """y ~= rsqrt(src/D + eps) on DVE (Quake seed + 2 Newton iters)."""
            ms = stats.tile([128, w], _F32, tag=tagp + "ms")
            nc.vector.tensor_scalar(out=ms, in0=src_ap, scalar1=1.0 / D,
                                    scalar2=EPS, op0=OP.mult, op1=OP.add)
            i32 = stats.tile([128, w], _I32, tag=tagp + "i")
            nc.vector.tensor_scalar(out=i32, in0=ms.bitcast(_I32), scalar1=1,
                                    scalar2=-1, op0=OP.logical_shift_right,
                                    op1=OP.bitwise_xor)
            y0i = stats.tile([128, w], _I32, tag=tagp + "y0")
            nc.vector.tensor_scalar(out=y0i, in0=i32, scalar1=1597463008,
                                    scalar2=None, op0=OP.add)
            ycur = y0i.bitcast(_F32)
            t1 = stats.tile([128, w], _F32, tag=tagp + "t")
            for it in range(2):
                ynext = stats.tile([128, w], _F32, tag=tagp + f"y{it}")
                nc.vector.tensor_mul(out=t1, in0=ycur, in1=ycur)
                nc.vector.tensor_mul(out=t1, in0=t1, in1=ms)
                nc.vector.tensor_scalar(out=t1, in0=t1, scalar1=-0.5,
                                        scalar2=1.5, op0=OP.mult, op1=OP.add)
                nc.vector.tensor_mul(out=ynext, in0=ycur, in1=t1)
                ycur = ynext
            return ycur

        for tt in range(NT):
            v_sb = vin.tile([128, N_BLK, D], _BF)
            nc.sync.dma_start(out=v_sb, in_=vn[tt])
            vt_sb = vin.tile([128, N_BLK, NCH, 128], _BF)
            nc.sync.dma_start(out=vt_sb, in_=vt[tt])
            p_sb = vin.tile([128, D], _F32)
            nc.sync.dma_start(out=p_sb, in_=pf[tt])
            pb_sb = vin.tile([128, D], _BF)
            nc.sync.dma_start(out=pb_sb, in_=pb[tt])
            pt_sb = vin.tile([128, NCH, 128], _BF)
            nc.sync.dma_start_transpose(pt_sb, pb[tt])

            # --- dots for all 9 blocks x both queries (PE) -------------------
            d_ps = pdot.tile([128, 18], _F32)
            for n in range(9):
                for c in range(NCH):
                    lhsT = vt_sb[:, n, c, :] if n < 8 else pt_sb[:, c, :]
                    nc.tensor.matmul(d_ps[:, 2 * n:2 * n + 2], lhsT=lhsT,
                                     rhs=qp_sb[:, c, :],
                                     start=(c == 0), stop=(c == NCH - 1))
            dots = stats.tile([128, 18], _F32)
            nc.vector.tensor_copy(out=dots, in_=d_ps)
            dots_v = dots.rearrange("p (n j) -> p j n", j=2)

            # --- sum of squares (ACT square + accumulate) --------------------
            ssq = stats.tile([128, 9], _F32)
            junk = work.tile([128, D], _BF, tag="junk")
            for n in range(N_BLK):
                nc.scalar.activation(out=junk, in_=v_sb[:, n, :], func=AF.Square,
                                     accum_out=ssq[:, n:n + 1])
            nc.scalar.activation(out=junk, in_=pb_sb, func=AF.Square,
                                 accum_out=ssq[:, 8:9])

            pcur_bf = pb_sb
            pcur_f32 = p_sb
            for phase in range(2):
                w_sb = wa_sb if phase == 0 else wm_sb

                # logits = dot * rsqrt(ssq/D + eps); softmax over the 9 blocks
                rinv = rsqrt_dve(ssq[:, :], 9, "r9")
                lg = stats.tile([128, 9], _F32, tag="lg")
                nc.vector.tensor_mul(out=lg, in0=dots_v[:, phase, :], in1=rinv)
                mx = stats.tile([128, 1], _F32, tag="mx")
                nc.vector.reduce_max(out=mx, in_=lg, axis=AX.X)
                mb = stats.tile([128, 1], _F32, tag="mb")
                nc.vector.tensor_scalar_mul(out=mb, in0=mx, scalar1=-INV_SCALE)
                ex = stats.tile([128, 9], _F32, tag="ex")
                se = stats.tile([128, 1], _F32, tag="se")
                nc.scalar.activation(out=ex, in_=lg, func=AF.Exp,
                                     scale=INV_SCALE, bias=mb[:, :], accum_out=se)
                rs = stats.tile([128, 1], _F32, tag="rs")
                nc.vector.reciprocal(out=rs, in_=se)
                alpha = stats.tile([128, 9], _F32, tag="alpha")
                nc.vector.tensor_scalar_mul(out=alpha, in0=ex, scalar1=rs)

                # h = sum_n alpha_n * V_n via diag(alpha) matmuls
                diag = work.tile([128, 9, 128], _BF, tag="diag")
                for n in range(9):
                    nc.vector.tensor_scalar_mul(out=diag[:, n, :], in0=ident,
                                                scalar1=alpha[:, n:n + 1])
                h_ps = pbig.tile([128, D], _F32, tag="big")
                for half in range(2):
                    sl = slice(512 * half, 512 * half + 512)
                    for n in range(9):
                        rhs = v_sb[:, n, sl] if n < 8 else pcur_bf[:, sl]
                        nc.tensor.matmul(h_ps[:, sl], lhsT=diag[:, n, :], rhs=rhs,
                                         start=(n == 0), stop=(n == 8))

                # rmsnorm(h) -> hn (bf16), then transpose for the GEMM
                ssqh = stats.tile([128, 1], _F32, tag="ssqh")
                junk2 = work.tile([128, D], _BF, tag="junk")
                nc.scalar.activation(out=junk2, in_=h_ps, func=AF.Square,
                                     accum_out=ssqh)
                rih = rsqrt_dve(ssqh[:, :], 1, "rh")
                hn = work.tile([128, D], _BF, tag="hn")
                nc.scalar.activation(out=hn, in_=h_ps, func=AF.Copy,
                                     scale=rih[:, :])
                hnT = work.tile([128, NCH, 128], _BF, tag="hnT")
                nc.sync.dma_start_transpose(hnT, hn)

                # attn_out = hn @ W_eff.T ; out = partial + attn_out
                g_ps = pbig.tile([128, D], _F32, tag="big")
                for half in range(2):
                    sl = slice(512 * half, 512 * half + 512)
                    for c in range(NCH):
                        nc.tensor.matmul(g_ps[:, sl], lhsT=hnT[:, c, :],
                                         rhs=w_sb[:, c, sl],
                                         start=(c == 0), stop=(c == NCH - 1))
                pout = work.tile([128, D], _F32, tag="pout")
                nc.vector.tensor_add(out=pout, in0=g_ps, in1=pcur_f32)

                if phase == 0:
                    p1_bf = work.tile([128, D], _BF, tag="p1bf")
                    nc.vector.tensor_copy(out=p1_bf, in_=pout)
                    # phase-2 stats for the updated partial block
                    prod = work.tile([128, D], _BF, tag="prod")
                    nc.vector.tensor_mul(out=prod, in0=p1_bf, in1=qm_bc)
                    nc.vector.tensor_reduce(out=dots[:, 17:18], in_=prod,
                                            axis=AX.X, op=OP.add)
                    junk3 = work.tile([128, D], _BF, tag="junk")
                    nc.scalar.activation(out=junk3, in_=p1_bf, func=AF.Square,
                                         accum_out=ssq[:, 8:9])
                    pcur_bf = p1_bf
                    pcur_f32 = pout
                else:
                    nc.sync.dma_start(out=out[tt], in_=pout)

    nc.compile()
    return nc


def _get_nc():
    if "nc" not in _CACHE:
        _CACHE["nc"] = build_nc()
    return _CACHE["nc"]


def _prepare_in_maps(completed_blocks, partial_block, attn_norm_w, attn_w,
                     mlp_norm_w, mlp_w, attn_res_query, attn_res_norm_w,
                     mlp_res_query, mlp_res_norm_w):
    V = np.ascontiguousarray(np.asarray(completed_blocks, np.float32)).reshape(N_BLK, TOK, D)
    P = np.ascontiguousarray(np.asarray(partial_block, np.float32)).reshape(TOK, D)
    qwa = np.asarray(attn_res_query, np.float32) * np.asarray(attn_res_norm_w, np.float32)
    qwm = np.asarray(mlp_res_query, np.float32) * np.asarray(mlp_res_norm_w, np.float32)
    WaT = (np.asarray(attn_w, np.float32) * np.asarray(attn_norm_w, np.float32)[None, :]).T
    WmT = (np.asarray(mlp_w, np.float32) * np.asarray(mlp_norm_w, np.float32)[None, :]).T

    qp_host = np.ascontiguousarray(np.stack(
        [qwa.astype(bf16).reshape(NCH, 128).T, qwm.astype(bf16).reshape(NCH, 128).T],
        axis=-1))                                             # [p, c, 2]
    qm_host = np.ascontiguousarray(qwm.astype(bf16))
    wa_host = np.ascontiguousarray(WaT.astype(bf16).reshape(NCH, 128, D).transpose(1, 0, 2))
    wm_host = np.ascontiguousarray(WmT.astype(bf16).reshape(NCH, 128, D).transpose(1, 0, 2))

    in_maps = []
    for c in range(NCORES):
        sl = slice(c * TPC, (c + 1) * TPC)
        Vc = V[:, sl, :].astype(bf16)                          # [n, 1024, 1024]
        vn_host = np.ascontiguousarray(
            Vc.reshape(N_BLK, NT, 128, D).transpose(1, 2, 0, 3))          # [tt,t,n,d]
        vt_host = np.ascontiguousarray(
            Vc.reshape(N_BLK, NT, 128, NCH, 128).transpose(1, 4, 0, 3, 2))  # [tt,p,n,c,t]
        pf_host = np.ascontiguousarray(P[sl].reshape(NT, 128, D))
        pb_host = pf_host.astype(bf16)
        in_maps.append(dict(vn=vn_host, vt=vt_host, pf=pf_host, pb=pb_host,
                            qp=qp_host, qm=qm_host, wa=wa_host, wm=wm_host))
    return in_maps


def _run(in_maps, **kw):
    nc = _get_nc()
    return run_bass_kernel_spmd(nc, in_maps, core_ids=list(range(NCORES)), **kw)


def kernel(completed_blocks, partial_block, attn_norm_w, attn_w, mlp_norm_w,
           mlp_w, attn_res_query, attn_res_norm_w, mlp_res_query,
           mlp_res_norm_w, layer_in_block=None, **_unused):
    in_maps = _prepare_in_maps(completed_blocks, partial_block, attn_norm_w,
                               attn_w, mlp_norm_w, mlp_w, attn_res_query,
                               attn_res_norm_w, mlp_res_query, mlp_res_norm_w)
    res = _run(in_maps)
    outs = [np.asarray(r["out"], np.float32).reshape(TPC, D) for r in res.results]
    return np.concatenate(outs, axis=0).reshape(B, T, D)



# revision 4
# speedup vs baseline: 1.0654x; 1.0654x over previous
"""Trainium2 Bass kernel for nn_BlockAttnResTransformerBlock.

Two sequential inter-block-attention sub-layers over 9 blocks (8 completed +
partial).  Per token t (8192 tokens, data-parallel over 8 cores):
  logit_n = <q, V_n[t]> * rsqrt(mean(V_n[t]^2) + eps) / sqrt(D)
  w_n     = exp(logit_n)            (softmax normalizer skipped: rmsnorm of h
                                     is scale-invariant, so it cancels)
  h       = sum_n w_n V_n[t]
  out     = partial[t] + rmsnorm(h) @ W_eff.T    (W_eff = W * norm_w, folded)
Phase 2 repeats with the updated partial and the mlp query/weights.

Engine split (per 128-token tile):
  DVE: 18+1 fused dot-products (scalar_tensor_tensor with accum_out),
       reciprocal for rsqrt, small logit muls
  ACT: 9+1 sum-of-squares (Square + accum), exp, diag(w) builds, h-norm
  PE : diag h-build matmuls, the two DxD GEMMs, residual add folded into the
       GEMM PSUM accumulation via an identity matmul
V ships in ONE bf16 layout (natural [t, n, d], partial packed as block 8);
output is written f32 straight from PSUM.
"""

import numpy as np
import ml_dtypes
from contextlib import ExitStack

import concourse.bass as bass
import concourse.bacc as bacc
import concourse.tile as tile
from concourse import mybir
from concourse.bass_utils import run_bass_kernel_spmd
from concourse.masks import make_identity

bf16 = ml_dtypes.bfloat16

N_BLK = 8          # completed blocks
NB = 9             # + the partial block
B, T, D = 4, 2048, 1024
NCORES = 8
TOK = B * T                  # 8192
TPC = TOK // NCORES          # 1024 tokens per core
NT = TPC // 128              # 8 token-tiles per core
NCH = D // 128               # 8 d-chunks
EPS = 1e-6
INV_SCALE = 1.0 / 32.0       # 1/sqrt(D)

_BF = mybir.dt.bfloat16
_F32 = mybir.dt.float32

_CACHE = {}


def build_nc():
    nc = bacc.Bacc("TRN2", target_bir_lowering=False, debug=False)

    vn = nc.dram_tensor("vn", [NT, 128, NB, D], _BF, kind="ExternalInput")
    qb = nc.dram_tensor("qb", [2, D], _BF, kind="ExternalInput")
    wa = nc.dram_tensor("wa", [128, NCH, D], _BF, kind="ExternalInput")
    wm = nc.dram_tensor("wm", [128, NCH, D], _BF, kind="ExternalInput")
    out = nc.dram_tensor("out", [NT, 128, D], _F32, kind="ExternalOutput")

    AF = mybir.ActivationFunctionType
    OP = mybir.AluOpType

    with tile.TileContext(nc) as tc, ExitStack() as ctx:
        consts = ctx.enter_context(tc.tile_pool(name="consts", bufs=1))
        vin = ctx.enter_context(tc.tile_pool(name="vin", bufs=2))
        stats = ctx.enter_context(tc.tile_pool(name="stats", bufs=2))
        work = ctx.enter_context(tc.tile_pool(name="work", bufs=2))
        pbig = ctx.enter_context(tc.tile_pool(name="pbig", bufs=2, space="PSUM"))

        ident = consts.tile([128, 128], _BF)
        make_identity(nc, ident)
        eps_sb = consts.tile([128, 1], _F32)
        nc.vector.memset(eps_sb, EPS)
        qbc = consts.tile([128, 2, D], _BF)
        qb_ap = qb[:, :]
        nc.sync.dma_start(out=qbc, in_=bass.AP(
            tensor=qb_ap.tensor, offset=qb_ap.offset,
            ap=[[0, 128]] + list(qb_ap.ap)))
        wa_sb = consts.tile([128, NCH, D], _BF)
        nc.sync.dma_start(out=wa_sb, in_=wa[:, :, :])
        wm_sb = consts.tile([128, NCH, D], _BF)
        nc.sync.dma_start(out=wm_sb, in_=wm[:, :, :])

        for tt in range(NT):
            v = vin.tile([128, NB, D], _BF)
            nc.sync.dma_start(out=v, in_=vn[tt])

            ssq = stats.tile([128, NB], _F32)
            dots = stats.tile([128, 2, NB], _F32)

            # per-block reductions: ssq on ACT, dots on DVE (fused mul+sum)
            for n in range(NB):
                ja = work.tile([128, D], _BF, tag=f"ja{n % 2}")
                nc.scalar.activation(out=ja, in_=v[:, n, :], func=AF.Square,
                                     accum_out=ssq[:, n:n + 1])
            for qi in range(2):
                for n in range(NB):
                    jv = work.tile([128, D], _BF, tag=f"jv{n % 2}")
                    nc.vector.scalar_tensor_tensor(
                        out=jv, in0=v[:, n, :], scalar=1.0,
                        in1=qbc[:, qi, :], op0=OP.mult, op1=OP.mult,
                        accum_out=dots[:, qi, n:n + 1])

            pcur = v[:, 8, :]
            for phase in range(2):
                w_sb = wa_sb if phase == 0 else wm_sb

                # w_n = exp(dot_n * rsqrt(ssq_n/D + eps) / 32)
                sq = stats.tile([128, NB], _F32, tag=f"sq{phase}")
                nc.scalar.activation(out=sq, in_=ssq, func=AF.Sqrt,
                                     scale=1.0 / D, bias=eps_sb[:, :])
                rinv = stats.tile([128, NB], _F32, tag=f"ri{phase}")
                nc.vector.reciprocal_approx_fast(out=rinv, in_=sq)
                lg = stats.tile([128, NB], _F32, tag=f"lg{phase}")
                nc.vector.tensor_mul(out=lg, in0=dots[:, phase, :], in1=rinv)
                ex = stats.tile([128, NB], _F32, tag=f"ex{phase}")
                nc.scalar.activation(out=ex, in_=lg, func=AF.Exp,
                                     scale=INV_SCALE)

                # diag(w_n) on ACT, then h = sum_n w_n V_n on PE
                diag = work.tile([128, NB, 128], _BF, tag=f"dg{phase}")
                for n in range(NB):
                    nc.scalar.activation(out=diag[:, n, :], in_=ident,
                                         func=AF.Copy, scale=ex[:, n:n + 1])
                h_ps = pbig.tile([128, D], _F32, tag="h")
                for half in range(2):
                    hs = slice(512 * half, 512 * half + 512)
                    for n in range(NB):
                        rhs = v[:, n, hs] if n < 8 else pcur[:, hs]
                        nc.tensor.matmul(h_ps[:, hs], lhsT=diag[:, n, :],
                                         rhs=rhs, start=(n == 0),
                                         stop=(n == 8))

                # hn = h * rsqrt(mean(h^2) + eps), bf16, then transpose
                ssqh = stats.tile([128, 1], _F32, tag=f"sh{phase}")
                jh = work.tile([128, D], _BF, tag="jh")
                nc.scalar.activation(out=jh, in_=h_ps, func=AF.Square,
                                     accum_out=ssqh)
                sqh = stats.tile([128, 1], _F32, tag=f"sz{phase}")
                nc.scalar.activation(out=sqh, in_=ssqh, func=AF.Sqrt,
                                     scale=1.0 / D, bias=eps_sb[:, :])
                rih = stats.tile([128, 1], _F32, tag=f"rz{phase}")
                nc.vector.reciprocal_approx_fast(out=rih, in_=sqh)
                hn = work.tile([128, D], _BF, tag=f"hn{phase}")
                nc.scalar.activation(out=hn, in_=h_ps, func=AF.Copy,
                                     scale=rih[:, :])
                hnT = work.tile([128, NCH, 128], _BF, tag=f"ht{phase}")
                nc.sync.dma_start_transpose(hnT, hn)

                # out = hn @ W_eff.T + partial   (residual via identity matmul)
                g_ps = pbig.tile([128, D], _F32, tag="g")
                for half in range(2):
                    hs = slice(512 * half, 512 * half + 512)
                    for c in range(NCH):
                        nc.tensor.matmul(g_ps[:, hs], lhsT=hnT[:, c, :],
                                         rhs=w_sb[:, c, hs],
                                         start=(c == 0), stop=False)
                    nc.tensor.matmul(g_ps[:, hs], lhsT=ident,
                                     rhs=pcur[:, hs], start=False, stop=True)

                if phase == 0:
                    p1 = work.tile([128, D], _BF, tag="p1")
                    nc.scalar.activation(out=p1, in_=g_ps, func=AF.Copy)
                    # refresh block-8 stats for phase 2
                    ja = work.tile([128, D], _BF, tag="ja0")
                    nc.scalar.activation(out=ja, in_=p1, func=AF.Square,
                                         accum_out=ssq[:, 8:9])
                    jv = work.tile([128, D], _BF, tag="jv0")
                    nc.vector.scalar_tensor_tensor(
                        out=jv, in0=p1, scalar=1.0, in1=qbc[:, 1, :],
                        op0=OP.mult, op1=OP.mult,
                        accum_out=dots[:, 1, 8:9])
                    pcur = p1
                else:
                    pout = work.tile([128, D], _F32, tag="po")
                    nc.vector.tensor_copy(out=pout, in_=g_ps)
                    nc.sync.dma_start(out=out[tt], in_=pout)

    nc.compile()
    return nc


def _get_nc():
    if "nc" not in _CACHE:
        _CACHE["nc"] = build_nc()
    return _CACHE["nc"]


def _prepare_in_maps(completed_blocks, partial_block, attn_norm_w, attn_w,
                     mlp_norm_w, mlp_w, attn_res_query, attn_res_norm_w,
                     mlp_res_query, mlp_res_norm_w):
    V = np.ascontiguousarray(
        np.asarray(completed_blocks, np.float32)).reshape(N_BLK, TOK, D)
    P = np.ascontiguousarray(
        np.asarray(partial_block, np.float32)).reshape(TOK, D)
    qwa = np.asarray(attn_res_query, np.float32) * np.asarray(attn_res_norm_w, np.float32)
    qwm = np.asarray(mlp_res_query, np.float32) * np.asarray(mlp_res_norm_w, np.float32)
    WaT = (np.asarray(attn_w, np.float32) * np.asarray(attn_norm_w, np.float32)[None, :]).T
    WmT = (np.asarray(mlp_w, np.float32) * np.asarray(mlp_norm_w, np.float32)[None, :]).T

    qb_host = np.ascontiguousarray(
        np.stack([qwa, qwm], axis=0).astype(bf16))            # [2, D]
    wa_host = np.ascontiguousarray(
        WaT.astype(bf16).reshape(NCH, 128, D).transpose(1, 0, 2))
    wm_host = np.ascontiguousarray(
        WmT.astype(bf16).reshape(NCH, 128, D).transpose(1, 0, 2))

    in_maps = []
    for c in range(NCORES):
        sl = slice(c * TPC, (c + 1) * TPC)
        Vc = V[:, sl, :].astype(bf16)                          # [8, 1024, 1024]
        Pc = P[sl].astype(bf16)                                # [1024, 1024]
        vn_host = np.empty((NT, 128, NB, D), dtype=bf16)
        vn_host[:, :, :8, :] = Vc.reshape(N_BLK, NT, 128, D).transpose(1, 2, 0, 3)
        vn_host[:, :, 8, :] = Pc.reshape(NT, 128, D)
        in_maps.append(dict(vn=vn_host, qb=qb_host, wa=wa_host, wm=wm_host))
    return in_maps


def _run(in_maps, **kw):
    nc = _get_nc()
    return run_bass_kernel_spmd(nc, in_maps, core_ids=list(range(NCORES)), **kw)


def kernel(completed_blocks, partial_block, attn_norm_w, attn_w, mlp_norm_w,
           mlp_w, attn_res_query, attn_res_norm_w, mlp_res_query,
           mlp_res_norm_w, layer_in_block=None, **_unused):
    in_maps = _prepare_in_maps(completed_blocks, partial_block, attn_norm_w,
                               attn_w, mlp_norm_w, mlp_w, attn_res_query,
                               attn_res_norm_w, mlp_res_query, mlp_res_norm_w)
    res = _run(in_maps)
    outs = [np.asarray(r["out"], np.float32).reshape(TPC, D) for r in res.results]
    return np.concatenate(outs, axis=0).reshape(B, T, D)


# revision 5
# speedup vs baseline: 1.1950x; 1.1217x over previous
"""Trainium2 Bass kernel for nn_BlockAttnResTransformerBlock.

Two sequential inter-block-attention sub-layers over 9 blocks (8 completed +
partial).  Per token t (8192 tokens, data-parallel over 8 cores):
  logit_n = <q, V_n[t]> * rsqrt(mean(V_n[t]^2) + eps) / sqrt(D)
  w_n     = exp(logit_n)            (softmax normalizer skipped: rmsnorm of h
                                     is scale-invariant, so it cancels)
  h       = sum_n w_n V_n[t]
  out     = partial[t] + rmsnorm(h) @ W_eff.T    (W_eff = W * norm_w, folded)
Phase 2 repeats with the updated partial and the mlp query/weights.

Engine split (per 128-token tile):
  DVE: 18+1 fused dot-products (scalar_tensor_tensor with accum_out),
       reciprocal for rsqrt, small logit muls
  ACT: 9+1 sum-of-squares (Square + accum), exp, diag(w) builds, h-norm
  PE : diag h-build matmuls, the two DxD GEMMs, residual add folded into the
       GEMM PSUM accumulation via an identity matmul
V ships in ONE bf16 layout (natural [t, n, d], partial packed as block 8);
output is written f32 straight from PSUM.
"""

import numpy as np
import ml_dtypes
from contextlib import ExitStack

import concourse.bass as bass
import concourse.bacc as bacc
import concourse.tile as tile
from concourse import mybir
from concourse.bass_utils import run_bass_kernel_spmd
from concourse.masks import make_identity

bf16 = ml_dtypes.bfloat16

N_BLK = 8          # completed blocks
NB = 9             # + the partial block
B, T, D = 4, 2048, 1024
NCORES = 8
TOK = B * T                  # 8192
TPC = TOK // NCORES          # 1024 tokens per core
NT = TPC // 128              # 8 token-tiles per core
NCH = D // 128               # 8 d-chunks
EPS = 1e-6
INV_SCALE = 1.0 / 32.0       # 1/sqrt(D)

_BF = mybir.dt.bfloat16
_F32 = mybir.dt.float32

_CACHE = {}


def build_nc():
    nc = bacc.Bacc("TRN2", target_bir_lowering=False, debug=False)

    vn = nc.dram_tensor("vn", [NT, 128, NB, D], _BF, kind="ExternalInput")
    qb = nc.dram_tensor("qb", [2, D], _BF, kind="ExternalInput")
    wa = nc.dram_tensor("wa", [128, NCH, D], _BF, kind="ExternalInput")
    wm = nc.dram_tensor("wm", [128, NCH, D], _BF, kind="ExternalInput")
    out = nc.dram_tensor("out", [NT, 128, D], _F32, kind="ExternalOutput")

    AF = mybir.ActivationFunctionType
    OP = mybir.AluOpType

    with tile.TileContext(nc) as tc, ExitStack() as ctx:
        consts = ctx.enter_context(tc.tile_pool(name="consts", bufs=1))
        vin = ctx.enter_context(tc.tile_pool(name="vin", bufs=3))
        stats = ctx.enter_context(tc.tile_pool(name="stats", bufs=4))
        work = ctx.enter_context(tc.tile_pool(name="work", bufs=2))
        pbig = ctx.enter_context(tc.tile_pool(name="pbig", bufs=2, space="PSUM"))

        ident = consts.tile([128, 128], _BF)
        make_identity(nc, ident)
        eps_sb = consts.tile([128, 1], _F32)
        nc.vector.memset(eps_sb, EPS)
        qbc = consts.tile([128, 2, D], _BF)
        qb_ap = qb[:, :]
        nc.sync.dma_start(out=qbc, in_=bass.AP(
            tensor=qb_ap.tensor, offset=qb_ap.offset,
            ap=[[0, 128]] + list(qb_ap.ap)))
        wa_sb = consts.tile([128, NCH, D], _BF)
        nc.sync.dma_start(out=wa_sb, in_=wa[:, :, :])
        wm_sb = consts.tile([128, NCH, D], _BF)
        nc.sync.dma_start(out=wm_sb, in_=wm[:, :, :])

        _I32 = mybir.dt.int32

        def rsqrt_dve(src_ap, w, tag):
            """y ~= rsqrt(src/D + eps) on DVE (Quake seed + 2 Newton iters)."""
            ms = stats.tile([128, w], _F32, tag=tag + "ms")
            nc.vector.tensor_scalar(out=ms, in0=src_ap, scalar1=1.0 / D,
                                    scalar2=EPS, op0=OP.mult, op1=OP.add)
            i32 = stats.tile([128, w], _I32, tag=tag + "i")
            nc.vector.tensor_scalar(out=i32, in0=ms.bitcast(_I32), scalar1=1,
                                    scalar2=-1, op0=OP.logical_shift_right,
                                    op1=OP.bitwise_xor)
            y0i = stats.tile([128, w], _I32, tag=tag + "y0")
            nc.vector.tensor_scalar(out=y0i, in0=i32, scalar1=1597463008,
                                    scalar2=None, op0=OP.add)
            ycur = y0i.bitcast(_F32)
            t1 = stats.tile([128, w], _F32, tag=tag + "t")
            for it in range(2):
                ynext = stats.tile([128, w], _F32, tag=tag + f"y{it}")
                nc.vector.tensor_mul(out=t1, in0=ycur, in1=ycur)
                nc.vector.tensor_mul(out=t1, in0=t1, in1=ms)
                nc.vector.tensor_scalar(out=t1, in0=t1, scalar1=-0.5,
                                        scalar2=1.5, op0=OP.mult, op1=OP.add)
                nc.vector.tensor_mul(out=ynext, in0=ycur, in1=t1)
                ycur = ynext
            return ycur

        for tt in range(NT):
            v = vin.tile([128, NB, D], _BF)
            nc.sync.dma_start(out=v, in_=vn[tt])

            ssq = stats.tile([128, NB], _F32)
            dots = stats.tile([128, 2, NB], _F32)

            # per-block reductions: ssq on ACT, dots on DVE (fused mul+sum)
            for n in range(NB):
                ja = work.tile([128, D], _BF, tag=f"ja{n % 2}")
                nc.scalar.activation(out=ja, in_=v[:, n, :], func=AF.Square,
                                     accum_out=ssq[:, n:n + 1])
            for qi in range(2):
                for n in range(NB):
                    jv = work.tile([128, D], _BF, tag=f"jv{n % 2}")
                    nc.vector.scalar_tensor_tensor(
                        out=jv, in0=v[:, n, :], scalar=1.0,
                        in1=qbc[:, qi, :], op0=OP.mult, op1=OP.mult,
                        accum_out=dots[:, qi, n:n + 1])

            pcur = v[:, 8, :]
            for phase in range(2):
                w_sb = wa_sb if phase == 0 else wm_sb

                # w_n = exp(dot_n * rsqrt(ssq_n/D + eps) / 32)
                rinv = rsqrt_dve(ssq[:, :], NB, f"r{phase}")
                lg = stats.tile([128, NB], _F32, tag=f"lg{phase}")
                nc.vector.tensor_mul(out=lg, in0=dots[:, phase, :], in1=rinv)
                ex = stats.tile([128, NB], _F32, tag=f"ex{phase}")
                nc.scalar.activation(out=ex, in_=lg, func=AF.Exp,
                                     scale=INV_SCALE)

                # diag(w_n) on ACT, then h = sum_n w_n V_n on PE
                diag = work.tile([128, NB, 128], _BF, tag=f"dg{phase}")
                for n in range(NB):
                    nc.scalar.activation(out=diag[:, n, :], in_=ident,
                                         func=AF.Copy, scale=ex[:, n:n + 1])
                h_ps = pbig.tile([128, D], _F32, tag="h")
                for half in range(2):
                    hs = slice(512 * half, 512 * half + 512)
                    for n in range(NB):
                        rhs = v[:, n, hs] if n < 8 else pcur[:, hs]
                        nc.tensor.matmul(h_ps[:, hs], lhsT=diag[:, n, :],
                                         rhs=rhs, start=(n == 0),
                                         stop=(n == 8))

                # cast h to bf16 unscaled; rmsnorm scale folds into the
                # post-GEMM fused op (GEMM is linear in h)
                ssqh = stats.tile([128, 1], _F32, tag=f"sh{phase}")
                jh = work.tile([128, D], _BF, tag="jh")
                nc.scalar.activation(out=jh, in_=h_ps, func=AF.Square,
                                     accum_out=ssqh)
                rih = rsqrt_dve(ssqh[:, :], 1, f"z{phase}")
                hn = work.tile([128, D], _BF, tag=f"hn{phase}")
                nc.scalar.activation(out=hn, in_=h_ps, func=AF.Copy)
                hnT = work.tile([128, NCH, 128], _BF, tag=f"ht{phase}")
                nc.sync.dma_start_transpose(hnT, hn)

                # g = hn_unscaled @ W_eff.T ; out = g * rih + partial
                g_ps = pbig.tile([128, D], _F32, tag="g")
                for half in range(2):
                    hs = slice(512 * half, 512 * half + 512)
                    for c in range(NCH):
                        nc.tensor.matmul(g_ps[:, hs], lhsT=hnT[:, c, :],
                                         rhs=w_sb[:, c, hs],
                                         start=(c == 0), stop=(c == NCH - 1))
                pout = work.tile([128, D], _F32, tag=f"po{phase}")
                nc.vector.scalar_tensor_tensor(
                    out=pout, in0=g_ps, scalar=rih[:, :], in1=pcur,
                    op0=OP.mult, op1=OP.add)

                if phase == 0:
                    p1 = work.tile([128, D], _BF, tag="p1")
                    nc.vector.tensor_copy(out=p1, in_=pout)
                    # refresh block-8 stats for phase 2
                    ja = work.tile([128, D], _BF, tag="ja0")
                    nc.scalar.activation(out=ja, in_=p1, func=AF.Square,
                                         accum_out=ssq[:, 8:9])
                    jv = work.tile([128, D], _BF, tag="jv0")
                    nc.vector.scalar_tensor_tensor(
                        out=jv, in0=p1, scalar=1.0, in1=qbc[:, 1, :],
                        op0=OP.mult, op1=OP.mult,
                        accum_out=dots[:, 1, 8:9])
                    pcur = p1
                else:
                    nc.sync.dma_start(out=out[tt], in_=pout)

    nc.compile()
    return nc


def _get_nc():
    if "nc" not in _CACHE:
        _CACHE["nc"] = build_nc()
    return _CACHE["nc"]


def _prepare_in_maps(completed_blocks, partial_block, attn_norm_w, attn_w,
                     mlp_norm_w, mlp_w, attn_res_query, attn_res_norm_w,
                     mlp_res_query, mlp_res_norm_w):
    V = np.ascontiguousarray(
        np.asarray(completed_blocks, np.float32)).reshape(N_BLK, TOK, D)
    P = np.ascontiguousarray(
        np.asarray(partial_block, np.float32)).reshape(TOK, D)
    qwa = np.asarray(attn_res_query, np.float32) * np.asarray(attn_res_norm_w, np.float32)
    qwm = np.asarray(mlp_res_query, np.float32) * np.asarray(mlp_res_norm_w, np.float32)
    WaT = (np.asarray(attn_w, np.float32) * np.asarray(attn_norm_w, np.float32)[None, :]).T
    WmT = (np.asarray(mlp_w, np.float32) * np.asarray(mlp_norm_w, np.float32)[None, :]).T

    qb_host = np.ascontiguousarray(
        np.stack([qwa, qwm], axis=0).astype(bf16))            # [2, D]
    wa_host = np.ascontiguousarray(
        WaT.astype(bf16).reshape(NCH, 128, D).transpose(1, 0, 2))
    wm_host = np.ascontiguousarray(
        WmT.astype(bf16).reshape(NCH, 128, D).transpose(1, 0, 2))

    in_maps = []
    for c in range(NCORES):
        sl = slice(c * TPC, (c + 1) * TPC)
        Vc = V[:, sl, :].astype(bf16)                          # [8, 1024, 1024]
        Pc = P[sl].astype(bf16)                                # [1024, 1024]
        vn_host = np.empty((NT, 128, NB, D), dtype=bf16)
        vn_host[:, :, :8, :] = Vc.reshape(N_BLK, NT, 128, D).transpose(1, 2, 0, 3)
        vn_host[:, :, 8, :] = Pc.reshape(NT, 128, D)
        in_maps.append(dict(vn=vn_host, qb=qb_host, wa=wa_host, wm=wm_host))
    return in_maps


def _run(in_maps, **kw):
    nc = _get_nc()
    return run_bass_kernel_spmd(nc, in_maps, core_ids=list(range(NCORES)), **kw)


def kernel(completed_blocks, partial_block, attn_norm_w, attn_w, mlp_norm_w,
           mlp_w, attn_res_query, attn_res_norm_w, mlp_res_query,
           mlp_res_norm_w, layer_in_block=None, **_unused):
    in_maps = _prepare_in_maps(completed_blocks, partial_block, attn_norm_w,
                               attn_w, mlp_norm_w, mlp_w, attn_res_query,
                               attn_res_norm_w, mlp_res_query, mlp_res_norm_w)
    res = _run(in_maps)
    outs = [np.asarray(r["out"], np.float32).reshape(TPC, D) for r in res.results]
    return np.concatenate(outs, axis=0).reshape(B, T, D)


# revision 6
# speedup vs baseline: 1.3301x; 1.1131x over previous
"""Trainium2 Bass kernel for nn_BlockAttnResTransformerBlock.

Two sequential inter-block-attention sub-layers over 9 blocks (8 completed +
partial).  Per token t (8192 tokens, data-parallel over 8 cores):
  logit_n = <q, V_n[t]> * rsqrt(mean(V_n[t]^2) + eps) / sqrt(D)
  w_n     = exp(logit_n)            (softmax normalizer skipped: rmsnorm of h
                                     is scale-invariant, so it cancels)
  h       = sum_n w_n V_n[t]
  out     = partial[t] + rmsnorm(h) @ W_eff.T    (W_eff = W * norm_w, folded)
Phase 2 repeats with the updated partial and the mlp query/weights.

Engine split (per 128-token tile):
  DVE: 18+1 fused dot-products (scalar_tensor_tensor with accum_out),
       reciprocal for rsqrt, small logit muls
  ACT: 9+1 sum-of-squares (Square + accum), exp, diag(w) builds, h-norm
  PE : diag h-build matmuls, the two DxD GEMMs, residual add folded into the
       GEMM PSUM accumulation via an identity matmul
V ships in ONE bf16 layout (natural [t, n, d], partial packed as block 8);
output is written f32 straight from PSUM.
"""

import numpy as np
import ml_dtypes
from contextlib import ExitStack

import concourse.bass as bass
import concourse.bacc as bacc
import concourse.tile as tile
from concourse import mybir
from concourse.bass_utils import run_bass_kernel_spmd
from concourse.masks import make_identity

bf16 = ml_dtypes.bfloat16

N_BLK = 8          # completed blocks
NB = 9             # + the partial block
B, T, D = 4, 2048, 1024
NCORES = 8
TOK = B * T                  # 8192
TPC = TOK // NCORES          # 1024 tokens per core
NT = TPC // 128              # 8 token-tiles per core
NCH = D // 128               # 8 d-chunks
EPS = 1e-6
INV_SCALE = 1.0 / 32.0       # 1/sqrt(D)

_BF = mybir.dt.bfloat16
_F32 = mybir.dt.float32

_CACHE = {}


def build_nc():
    nc = bacc.Bacc("TRN2", target_bir_lowering=False, debug=False)

    vn = nc.dram_tensor("vn", [NT, 128, NB, D], _BF, kind="ExternalInput")
    qb = nc.dram_tensor("qb", [2, D], _BF, kind="ExternalInput")
    wa = nc.dram_tensor("wa", [128, NCH, D], _BF, kind="ExternalInput")
    wm = nc.dram_tensor("wm", [128, NCH, D], _BF, kind="ExternalInput")
    out = nc.dram_tensor("out", [NT, 128, D], _F32, kind="ExternalOutput")

    AF = mybir.ActivationFunctionType
    OP = mybir.AluOpType

    with tile.TileContext(nc) as tc, ExitStack() as ctx:
        consts = ctx.enter_context(tc.tile_pool(name="consts", bufs=1))
        vin = ctx.enter_context(tc.tile_pool(name="vin", bufs=3))
        stats = ctx.enter_context(tc.tile_pool(name="stats", bufs=4))
        work = ctx.enter_context(tc.tile_pool(name="work", bufs=2))
        pbig = ctx.enter_context(tc.tile_pool(name="pbig", bufs=2, space="PSUM"))

        ident = consts.tile([128, 128], _BF)
        make_identity(nc, ident)
        ident9 = consts.tile([128, NB, 128], _BF)
        for n in range(NB):
            nc.vector.tensor_copy(out=ident9[:, n, :], in_=ident)
        eps_sb = consts.tile([128, 1], _F32)
        nc.vector.memset(eps_sb, EPS)
        qbc = consts.tile([128, 2, D], _BF)
        qb_ap = qb[:, :]
        nc.sync.dma_start(out=qbc, in_=bass.AP(
            tensor=qb_ap.tensor, offset=qb_ap.offset,
            ap=[[0, 128]] + list(qb_ap.ap)))
        wa_sb = consts.tile([128, NCH, D], _BF)
        nc.sync.dma_start(out=wa_sb, in_=wa[:, :, :])
        wm_sb = consts.tile([128, NCH, D], _BF)
        nc.sync.dma_start(out=wm_sb, in_=wm[:, :, :])

        _I32 = mybir.dt.int32

        def rsqrt_dve(src_ap, w, tag):
            """y ~= rsqrt(src/D + eps) on DVE (Quake seed + 2 Newton iters)."""
            ms = stats.tile([128, w], _F32, tag=tag + "ms")
            nc.vector.tensor_scalar(out=ms, in0=src_ap, scalar1=1.0 / D,
                                    scalar2=EPS, op0=OP.mult, op1=OP.add)
            i32 = stats.tile([128, w], _I32, tag=tag + "i")
            nc.vector.tensor_scalar(out=i32, in0=ms.bitcast(_I32), scalar1=1,
                                    scalar2=-1, op0=OP.logical_shift_right,
                                    op1=OP.bitwise_xor)
            y0i = stats.tile([128, w], _I32, tag=tag + "y0")
            nc.vector.tensor_scalar(out=y0i, in0=i32, scalar1=1597463008,
                                    scalar2=None, op0=OP.add)
            ycur = y0i.bitcast(_F32)
            t1 = stats.tile([128, w], _F32, tag=tag + "t")
            for it in range(1):
                ynext = stats.tile([128, w], _F32, tag=tag + f"y{it}")
                nc.vector.tensor_mul(out=t1, in0=ycur, in1=ycur)
                nc.vector.tensor_mul(out=t1, in0=t1, in1=ms)
                nc.vector.tensor_scalar(out=t1, in0=t1, scalar1=-0.5,
                                        scalar2=1.5, op0=OP.mult, op1=OP.add)
                nc.vector.tensor_mul(out=ynext, in0=ycur, in1=t1)
                ycur = ynext
            return ycur

        for tt in range(NT):
            v = vin.tile([128, NB, D], _BF)
            nc.sync.dma_start(out=v, in_=vn[tt])

            ssq = stats.tile([128, NB], _F32)
            dots = stats.tile([128, 2, NB], _F32)

            # per-block reductions: ssq on ACT, dots on DVE (fused mul+sum)
            for n in range(NB):
                ja = work.tile([128, D], _BF, tag=f"ja{n % 2}")
                nc.scalar.activation(out=ja, in_=v[:, n, :], func=AF.Square,
                                     accum_out=ssq[:, n:n + 1])
            for qi in range(2):
                for n in range(NB):
                    if (qi * NB + n) % 2 == 0:
                        jv = work.tile([128, D], _BF, tag=f"jv{n % 2}")
                        nc.vector.scalar_tensor_tensor(
                            out=jv, in0=v[:, n, :], scalar=1.0,
                            in1=qbc[:, qi, :], op0=OP.mult, op1=OP.mult,
                            accum_out=dots[:, qi, n:n + 1])
                    else:
                        pr = work.tile([128, D], _BF, tag=f"pr{n % 2}")
                        nc.vector.tensor_mul(out=pr, in0=v[:, n, :],
                                             in1=qbc[:, qi, :])
                        jb = work.tile([128, D], _BF, tag=f"jb{n % 2}")
                        nc.scalar.activation(out=jb, in_=pr, func=AF.Copy,
                                             accum_out=dots[:, qi, n:n + 1])

            pcur = v[:, 8, :]
            for phase in range(2):
                w_sb = wa_sb if phase == 0 else wm_sb

                # w_n = exp(dot_n * rsqrt(ssq_n/D + eps) / 32)
                rinv = rsqrt_dve(ssq[:, :], NB, f"r{phase}")
                lg = stats.tile([128, NB], _F32, tag=f"lg{phase}")
                nc.vector.tensor_mul(out=lg, in0=dots[:, phase, :], in1=rinv)
                ex = stats.tile([128, NB], _BF, tag=f"ex{phase}")
                nc.scalar.activation(out=ex, in_=lg, func=AF.Exp,
                                     scale=INV_SCALE)

                # diag(w_n) for all 9 blocks in one broadcast multiply
                diag = work.tile([128, NB, 128], _BF, tag=f"dg{phase}")
                ex_ap = ex[:, :]
                ex_bc = bass.AP(tensor=ex_ap.tensor, offset=ex_ap.offset,
                                ap=list(ex_ap.ap) + [[0, 128]])
                nc.vector.tensor_mul(out=diag, in0=ident9, in1=ex_bc)
                h_ps = pbig.tile([128, D], _F32, tag="h")
                for half in range(2):
                    hs = slice(512 * half, 512 * half + 512)
                    for n in range(NB):
                        rhs = v[:, n, hs] if n < 8 else pcur[:, hs]
                        nc.tensor.matmul(h_ps[:, hs], lhsT=diag[:, n, :],
                                         rhs=rhs, start=(n == 0),
                                         stop=(n == 8))

                # cast h to bf16 unscaled; rmsnorm scale folds into the
                # post-GEMM fused op (GEMM is linear in h)
                ssqh = stats.tile([128, 1], _F32, tag=f"sh{phase}")
                jh = work.tile([128, D], _BF, tag="jh")
                nc.scalar.activation(out=jh, in_=h_ps, func=AF.Square,
                                     accum_out=ssqh)
                rih = rsqrt_dve(ssqh[:, :], 1, f"z{phase}")
                hn = work.tile([128, D], _BF, tag=f"hn{phase}")
                nc.scalar.activation(out=hn, in_=h_ps, func=AF.Copy)
                hnT = work.tile([128, NCH, 128], _BF, tag=f"ht{phase}")
                nc.sync.dma_start_transpose(hnT, hn)

                # g = hn_unscaled @ W_eff.T ; out = g * rih + partial
                g_ps = pbig.tile([128, D], _F32, tag="g")
                for half in range(2):
                    hs = slice(512 * half, 512 * half + 512)
                    for c in range(NCH):
                        nc.tensor.matmul(g_ps[:, hs], lhsT=hnT[:, c, :],
                                         rhs=w_sb[:, c, hs],
                                         start=(c == 0), stop=(c == NCH - 1))
                pout = work.tile([128, D], _F32, tag=f"po{phase}")
                nc.vector.scalar_tensor_tensor(
                    out=pout, in0=g_ps, scalar=rih[:, :], in1=pcur,
                    op0=OP.mult, op1=OP.add)

                if phase == 0:
                    p1 = work.tile([128, D], _BF, tag="p1")
                    nc.vector.tensor_copy(out=p1, in_=pout)
                    # refresh block-8 stats for phase 2
                    ja = work.tile([128, D], _BF, tag="ja0")
                    nc.scalar.activation(out=ja, in_=p1, func=AF.Square,
                                         accum_out=ssq[:, 8:9])
                    jv = work.tile([128, D], _BF, tag="jv0")
                    nc.vector.scalar_tensor_tensor(
                        out=jv, in0=p1, scalar=1.0, in1=qbc[:, 1, :],
                        op0=OP.mult, op1=OP.mult,
                        accum_out=dots[:, 1, 8:9])
                    pcur = p1
                else:
                    nc.sync.dma_start(out=out[tt], in_=pout)

    nc.compile()
    return nc


def _get_nc():
    if "nc" not in _CACHE:
        _CACHE["nc"] = build_nc()
    return _CACHE["nc"]


def _prepare_in_maps(completed_blocks, partial_block, attn_norm_w, attn_w,
                     mlp_norm_w, mlp_w, attn_res_query, attn_res_norm_w,
                     mlp_res_query, mlp_res_norm_w):
    V = np.ascontiguousarray(
        np.asarray(completed_blocks, np.float32)).reshape(N_BLK, TOK, D)
    P = np.ascontiguousarray(
        np.asarray(partial_block, np.float32)).reshape(TOK, D)
    qwa = np.asarray(attn_res_query, np.float32) * np.asarray(attn_res_norm_w, np.float32)
    qwm = np.asarray(mlp_res_query, np.float32) * np.asarray(mlp_res_norm_w, np.float32)
    WaT = (np.asarray(attn_w, np.float32) * np.asarray(attn_norm_w, np.float32)[None, :]).T
    WmT = (np.asarray(mlp_w, np.float32) * np.asarray(mlp_norm_w, np.float32)[None, :]).T

    qb_host = np.ascontiguousarray(
        np.stack([qwa, qwm], axis=0).astype(bf16))            # [2, D]
    wa_host = np.ascontiguousarray(
        WaT.astype(bf16).reshape(NCH, 128, D).transpose(1, 0, 2))
    wm_host = np.ascontiguousarray(
        WmT.astype(bf16).reshape(NCH, 128, D).transpose(1, 0, 2))

    in_maps = []
    for c in range(NCORES):
        sl = slice(c * TPC, (c + 1) * TPC)
        Vc = V[:, sl, :].astype(bf16)                          # [8, 1024, 1024]
        Pc = P[sl].astype(bf16)                                # [1024, 1024]
        vn_host = np.empty((NT, 128, NB, D), dtype=bf16)
        vn_host[:, :, :8, :] = Vc.reshape(N_BLK, NT, 128, D).transpose(1, 2, 0, 3)
        vn_host[:, :, 8, :] = Pc.reshape(NT, 128, D)
        in_maps.append(dict(vn=vn_host, qb=qb_host, wa=wa_host, wm=wm_host))
    return in_maps


def _run(in_maps, **kw):
    nc = _get_nc()
    return run_bass_kernel_spmd(nc, in_maps, core_ids=list(range(NCORES)), **kw)


def kernel(completed_blocks, partial_block, attn_norm_w, attn_w, mlp_norm_w,
           mlp_w, attn_res_query, attn_res_norm_w, mlp_res_query,
           mlp_res_norm_w, layer_in_block=None, **_unused):
    in_maps = _prepare_in_maps(completed_blocks, partial_block, attn_norm_w,
                               attn_w, mlp_norm_w, mlp_w, attn_res_query,
                               attn_res_norm_w, mlp_res_query, mlp_res_norm_w)
    res = _run(in_maps)
    outs = [np.asarray(r["out"], np.float32).reshape(TPC, D) for r in res.results]
    return np.concatenate(outs, axis=0).reshape(B, T, D)


# revision 7
# speedup vs baseline: 1.3830x; 1.0397x over previous
"""Trainium2 Bass kernel for nn_BlockAttnResTransformerBlock.

Two sequential inter-block-attention sub-layers over 9 blocks (8 completed +
partial).  Per token t (8192 tokens, data-parallel over 8 cores):
  logit_n = <q, V_n[t]> * rsqrt(mean(V_n[t]^2) + eps) / sqrt(D)
  w_n     = exp(logit_n)            (softmax normalizer skipped: rmsnorm of h
                                     is scale-invariant, so it cancels)
  h       = sum_n w_n V_n[t]
  out     = partial[t] + rmsnorm(h) @ W_eff.T    (W_eff = W * norm_w, folded)
Phase 2 repeats with the updated partial and the mlp query/weights.

Engine split (per 128-token tile):
  DVE: 18+1 fused dot-products (scalar_tensor_tensor with accum_out),
       reciprocal for rsqrt, small logit muls
  ACT: 9+1 sum-of-squares (Square + accum), exp, diag(w) builds, h-norm
  PE : diag h-build matmuls, the two DxD GEMMs, residual add folded into the
       GEMM PSUM accumulation via an identity matmul
V ships in ONE bf16 layout (natural [t, n, d], partial packed as block 8);
output is written f32 straight from PSUM.
"""

import numpy as np
import ml_dtypes
from contextlib import ExitStack

import concourse.bass as bass
import concourse.bacc as bacc
import concourse.tile as tile
from concourse import mybir
from concourse.bass_utils import run_bass_kernel_spmd
from concourse.masks import make_identity

bf16 = ml_dtypes.bfloat16

N_BLK = 8          # completed blocks
NB = 9             # + the partial block
B, T, D = 4, 2048, 1024
NCORES = 8
TOK = B * T                  # 8192
TPC = TOK // NCORES          # 1024 tokens per core
NT = TPC // 128              # 8 token-tiles per core
NCH = D // 128               # 8 d-chunks
EPS = 1e-6
INV_SCALE = 1.0 / 32.0       # 1/sqrt(D)

_BF = mybir.dt.bfloat16
_F32 = mybir.dt.float32

_CACHE = {}


def build_nc():
    nc = bacc.Bacc("TRN2", target_bir_lowering=False, debug=False)

    vn = nc.dram_tensor("vn", [NT, 128, NB, D], _BF, kind="ExternalInput")
    qb = nc.dram_tensor("qb", [2, D], _BF, kind="ExternalInput")
    wa = nc.dram_tensor("wa", [128, NCH, D], _BF, kind="ExternalInput")
    wm = nc.dram_tensor("wm", [128, NCH, D], _BF, kind="ExternalInput")
    out = nc.dram_tensor("out", [NT, 128, D], _F32, kind="ExternalOutput")

    AF = mybir.ActivationFunctionType
    OP = mybir.AluOpType

    with tile.TileContext(nc) as tc, ExitStack() as ctx:
        consts = ctx.enter_context(tc.tile_pool(name="consts", bufs=1))
        vin = ctx.enter_context(tc.tile_pool(name="vin", bufs=3))
        stats = ctx.enter_context(tc.tile_pool(name="stats", bufs=4))
        work = ctx.enter_context(tc.tile_pool(name="work", bufs=2))
        pbig = ctx.enter_context(tc.tile_pool(name="pbig", bufs=2, space="PSUM"))

        ident = consts.tile([128, 128], _BF)
        make_identity(nc, ident)
        ident9 = consts.tile([128, NB, 128], _BF)
        for n in range(NB):
            nc.vector.tensor_copy(out=ident9[:, n, :], in_=ident)
        eps_sb = consts.tile([128, 1], _F32)
        nc.vector.memset(eps_sb, EPS)
        qbc = consts.tile([128, 2, D], _BF)
        qb_ap = qb[:, :]
        nc.sync.dma_start(out=qbc, in_=bass.AP(
            tensor=qb_ap.tensor, offset=qb_ap.offset,
            ap=[[0, 128]] + list(qb_ap.ap)))
        wa_sb = consts.tile([128, NCH, D], _BF)
        nc.sync.dma_start(out=wa_sb, in_=wa[:, :, :])
        wm_sb = consts.tile([128, NCH, D], _BF)
        nc.sync.dma_start(out=wm_sb, in_=wm[:, :, :])

        _I32 = mybir.dt.int32

        def rsqrt_dve(src_ap, w, tag):
            """y ~= rsqrt(src/D + eps) on DVE (Quake seed + 2 Newton iters)."""
            ms = stats.tile([128, w], _F32, tag=tag + "ms")
            nc.vector.tensor_scalar(out=ms, in0=src_ap, scalar1=1.0 / D,
                                    scalar2=EPS, op0=OP.mult, op1=OP.add)
            i32 = stats.tile([128, w], _I32, tag=tag + "i")
            nc.vector.tensor_scalar(out=i32, in0=ms.bitcast(_I32), scalar1=1,
                                    scalar2=-1, op0=OP.logical_shift_right,
                                    op1=OP.bitwise_xor)
            y0i = stats.tile([128, w], _I32, tag=tag + "y0")
            nc.vector.tensor_scalar(out=y0i, in0=i32, scalar1=1597463008,
                                    scalar2=None, op0=OP.add)
            ycur = y0i.bitcast(_F32)
            t1 = stats.tile([128, w], _F32, tag=tag + "t")
            for it in range(1):
                ynext = stats.tile([128, w], _F32, tag=tag + f"y{it}")
                nc.vector.tensor_mul(out=t1, in0=ycur, in1=ycur)
                nc.vector.tensor_mul(out=t1, in0=t1, in1=ms)
                nc.vector.tensor_scalar(out=t1, in0=t1, scalar1=-0.5,
                                        scalar2=1.5, op0=OP.mult, op1=OP.add)
                nc.vector.tensor_mul(out=ynext, in0=ycur, in1=t1)
                ycur = ynext
            return ycur

        state = {}

        def emit_stats(tt):
            v = vin.tile([128, NB, D], _BF, tag="v")
            nc.sync.dma_start(out=v, in_=vn[tt])
            ssq = stats.tile([128, NB], _F32, tag="ssq")
            dots = stats.tile([128, 2, NB], _F32, tag="dots")

            # per-block reductions: ssq on ACT, dots split DVE/ACT
            for n in range(NB):
                ja = work.tile([128, D], _BF, tag=f"ja{n % 2}")
                nc.scalar.activation(out=ja, in_=v[:, n, :], func=AF.Square,
                                     accum_out=ssq[:, n:n + 1])
            for qi in range(2):
                for n in range(NB):
                    if (qi * NB + n) % 2 == 0:
                        jv = work.tile([128, D], _BF, tag=f"jv{n % 2}")
                        nc.vector.scalar_tensor_tensor(
                            out=jv, in0=v[:, n, :], scalar=1.0,
                            in1=qbc[:, qi, :], op0=OP.mult, op1=OP.mult,
                            accum_out=dots[:, qi, n:n + 1])
                    else:
                        pr = work.tile([128, D], _BF, tag=f"pr{n % 2}")
                        nc.vector.tensor_mul(out=pr, in0=v[:, n, :],
                                             in1=qbc[:, qi, :])
                        jb = work.tile([128, D], _BF, tag=f"jb{n % 2}")
                        nc.scalar.activation(out=jb, in_=pr, func=AF.Copy,
                                             accum_out=dots[:, qi, n:n + 1])
            state[tt] = dict(v=v, ssq=ssq, dots=dots, pcur=v[:, 8, :])

        def emit_phase(tt, phase):
            st = state[tt]
            v, ssq, dots, pcur = st["v"], st["ssq"], st["dots"], st["pcur"]
            w_sb = wa_sb if phase == 0 else wm_sb

            # w_n = exp(dot_n * rsqrt(ssq_n/D + eps) / 32)
            rinv = rsqrt_dve(ssq[:, :], NB, f"r{phase}")
            lg = stats.tile([128, NB], _F32, tag=f"lg{phase}")
            nc.vector.tensor_mul(out=lg, in0=dots[:, phase, :], in1=rinv)
            ex = stats.tile([128, NB], _BF, tag=f"ex{phase}")
            nc.scalar.activation(out=ex, in_=lg, func=AF.Exp,
                                 scale=INV_SCALE)

            # diag(w_n) for all 9 blocks in one broadcast multiply
            diag = work.tile([128, NB, 128], _BF, tag=f"dg{phase}")
            ex_ap = ex[:, :]
            ex_bc = bass.AP(tensor=ex_ap.tensor, offset=ex_ap.offset,
                            ap=list(ex_ap.ap) + [[0, 128]])
            nc.vector.tensor_mul(out=diag, in0=ident9, in1=ex_bc)
            h_ps = pbig.tile([128, D], _F32, tag="h")
            for half in range(2):
                hs = slice(512 * half, 512 * half + 512)
                for n in range(NB):
                    rhs = v[:, n, hs] if n < 8 else pcur[:, hs]
                    nc.tensor.matmul(h_ps[:, hs], lhsT=diag[:, n, :],
                                     rhs=rhs, start=(n == 0), stop=(n == 8))

            # cast h to bf16 unscaled; rmsnorm scale folds into the
            # post-GEMM fused op (GEMM is linear in h)
            ssqh = stats.tile([128, 1], _F32, tag=f"sh{phase}")
            jh = work.tile([128, D], _BF, tag="jh")
            nc.scalar.activation(out=jh, in_=h_ps, func=AF.Square,
                                 accum_out=ssqh)
            rih = rsqrt_dve(ssqh[:, :], 1, f"z{phase}")
            hn = work.tile([128, D], _BF, tag=f"hn{phase}")
            nc.scalar.activation(out=hn, in_=h_ps, func=AF.Copy)
            hnT = work.tile([128, NCH, 128], _BF, tag=f"ht{phase}")
            nc.sync.dma_start_transpose(hnT, hn)

            # g = hn_unscaled @ W_eff.T ; out = g * rih + partial
            g_ps = pbig.tile([128, D], _F32, tag="g")
            for half in range(2):
                hs = slice(512 * half, 512 * half + 512)
                for c in range(NCH):
                    nc.tensor.matmul(g_ps[:, hs], lhsT=hnT[:, c, :],
                                     rhs=w_sb[:, c, hs],
                                     start=(c == 0), stop=(c == NCH - 1))
            pout = work.tile([128, D], _F32, tag=f"po{phase}")
            nc.vector.scalar_tensor_tensor(
                out=pout, in0=g_ps, scalar=rih[:, :], in1=pcur,
                op0=OP.mult, op1=OP.add)

            if phase == 0:
                p1 = work.tile([128, D], _BF, tag="p1")
                nc.vector.tensor_copy(out=p1, in_=pout)
                # refresh block-8 stats for phase 2
                ja = work.tile([128, D], _BF, tag="ja0")
                nc.scalar.activation(out=ja, in_=p1, func=AF.Square,
                                     accum_out=ssq[:, 8:9])
                jv = work.tile([128, D], _BF, tag="jv0")
                nc.vector.scalar_tensor_tensor(
                    out=jv, in0=p1, scalar=1.0, in1=qbc[:, 1, :],
                    op0=OP.mult, op1=OP.mult,
                    accum_out=dots[:, 1, 8:9])
                st["pcur"] = p1
            else:
                nc.sync.dma_start(out=out[tt], in_=pout)
                del state[tt]

        # software pipeline: stats(i) | phase1(i-1) | phase2(i-2)
        for i in range(NT + 2):
            if i < NT:
                emit_stats(i)
            if 1 <= i <= NT:
                emit_phase(i - 1, 0)
            if 2 <= i:
                emit_phase(i - 2, 1)

    nc.compile()
    return nc


def _get_nc():
    if "nc" not in _CACHE:
        _CACHE["nc"] = build_nc()
    return _CACHE["nc"]


def _prepare_in_maps(completed_blocks, partial_block, attn_norm_w, attn_w,
                     mlp_norm_w, mlp_w, attn_res_query, attn_res_norm_w,
                     mlp_res_query, mlp_res_norm_w):
    V = np.ascontiguousarray(
        np.asarray(completed_blocks, np.float32)).reshape(N_BLK, TOK, D)
    P = np.ascontiguousarray(
        np.asarray(partial_block, np.float32)).reshape(TOK, D)
    qwa = np.asarray(attn_res_query, np.float32) * np.asarray(attn_res_norm_w, np.float32)
    qwm = np.asarray(mlp_res_query, np.float32) * np.asarray(mlp_res_norm_w, np.float32)
    WaT = (np.asarray(attn_w, np.float32) * np.asarray(attn_norm_w, np.float32)[None, :]).T
    WmT = (np.asarray(mlp_w, np.float32) * np.asarray(mlp_norm_w, np.float32)[None, :]).T

    qb_host = np.ascontiguousarray(
        np.stack([qwa, qwm], axis=0).astype(bf16))            # [2, D]
    wa_host = np.ascontiguousarray(
        WaT.astype(bf16).reshape(NCH, 128, D).transpose(1, 0, 2))
    wm_host = np.ascontiguousarray(
        WmT.astype(bf16).reshape(NCH, 128, D).transpose(1, 0, 2))

    in_maps = []
    for c in range(NCORES):
        sl = slice(c * TPC, (c + 1) * TPC)
        Vc = V[:, sl, :].astype(bf16)                          # [8, 1024, 1024]
        Pc = P[sl].astype(bf16)                                # [1024, 1024]
        vn_host = np.empty((NT, 128, NB, D), dtype=bf16)
        vn_host[:, :, :8, :] = Vc.reshape(N_BLK, NT, 128, D).transpose(1, 2, 0, 3)
        vn_host[:, :, 8, :] = Pc.reshape(NT, 128, D)
        in_maps.append(dict(vn=vn_host, qb=qb_host, wa=wa_host, wm=wm_host))
    return in_maps


def _run(in_maps, **kw):
    nc = _get_nc()
    return run_bass_kernel_spmd(nc, in_maps, core_ids=list(range(NCORES)), **kw)


def kernel(completed_blocks, partial_block, attn_norm_w, attn_w, mlp_norm_w,
           mlp_w, attn_res_query, attn_res_norm_w, mlp_res_query,
           mlp_res_norm_w, layer_in_block=None, **_unused):
    in_maps = _prepare_in_maps(completed_blocks, partial_block, attn_norm_w,
                               attn_w, mlp_norm_w, mlp_w, attn_res_query,
                               attn_res_norm_w, mlp_res_query, mlp_res_norm_w)
    res = _run(in_maps)
    outs = [np.asarray(r["out"], np.float32).reshape(TPC, D) for r in res.results]
    return np.concatenate(outs, axis=0).reshape(B, T, D)


# revision 8
# speedup vs baseline: 1.7504x; 1.2657x over previous
"""Trainium2 Bass kernel for nn_BlockAttnResTransformerBlock.

Two sequential inter-block-attention sub-layers over 9 blocks (8 completed +
partial).  Per token t (8192 tokens, data-parallel over 8 cores):
  logit_n = <q, V_n[t]> * rsqrt(mean(V_n[t]^2) + eps) / sqrt(D)
  w_n     = exp(logit_n)            (softmax normalizer skipped: rmsnorm of h
                                     is scale-invariant, so it cancels)
  h       = sum_n w_n V_n[t]
  out     = partial[t] + rmsnorm(h) @ W_eff.T    (W_eff = W * norm_w, folded)
Phase 2 repeats with the updated partial and the mlp query/weights.

Engine split (per 128-token tile):
  DVE: 18+1 fused dot-products (scalar_tensor_tensor with accum_out),
       reciprocal for rsqrt, small logit muls
  ACT: 9+1 sum-of-squares (Square + accum), exp, diag(w) builds, h-norm
  PE : diag h-build matmuls, the two DxD GEMMs, residual add folded into the
       GEMM PSUM accumulation via an identity matmul
V ships in ONE bf16 layout (natural [t, n, d], partial packed as block 8);
output is written f32 straight from PSUM.
"""

import numpy as np
import ml_dtypes
from contextlib import ExitStack

import concourse.bass as bass
import concourse.bacc as bacc
import concourse.tile as tile
from concourse import mybir
from concourse.bass_utils import run_bass_kernel_spmd
from concourse.masks import make_identity

bf16 = ml_dtypes.bfloat16

N_BLK = 8          # completed blocks
NB = 9             # + the partial block
B, T, D = 4, 2048, 1024
NCORES = 8
TOK = B * T                  # 8192
TPC = TOK // NCORES          # 1024 tokens per core
NT = TPC // 128              # 8 token-tiles per core
NCH = D // 128               # 8 d-chunks
EPS = 1e-6
INV_SCALE = 1.0 / 32.0       # 1/sqrt(D)

_BF = mybir.dt.bfloat16
_F32 = mybir.dt.float32

_CACHE = {}


def build_nc():
    nc = bacc.Bacc("TRN2", target_bir_lowering=False, debug=False)

    vn = nc.dram_tensor("vn", [NT, 128, NB, D], _BF, kind="ExternalInput")
    qb = nc.dram_tensor("qb", [2, D], _BF, kind="ExternalInput")
    wa = nc.dram_tensor("wa", [128, NCH, D], _BF, kind="ExternalInput")
    wm = nc.dram_tensor("wm", [128, NCH, D], _BF, kind="ExternalInput")
    out = nc.dram_tensor("out", [NT, 128, D], _F32, kind="ExternalOutput")

    AF = mybir.ActivationFunctionType
    OP = mybir.AluOpType

    with tile.TileContext(nc) as tc, ExitStack() as ctx:
        consts = ctx.enter_context(tc.tile_pool(name="consts", bufs=1))
        vin = ctx.enter_context(tc.tile_pool(name="vin", bufs=3))
        stats = ctx.enter_context(tc.tile_pool(name="stats", bufs=4))
        work = ctx.enter_context(tc.tile_pool(name="work", bufs=2))
        pbig = ctx.enter_context(tc.tile_pool(name="pbig", bufs=2, space="PSUM"))

        ident = consts.tile([128, 128], _BF)
        make_identity(nc, ident)
        ident9 = consts.tile([128, NB, 128], _BF)
        for n in range(NB):
            nc.vector.tensor_copy(out=ident9[:, n, :], in_=ident)
        eps_sb = consts.tile([128, 1], _F32)
        nc.vector.memset(eps_sb, EPS)
        qbc = consts.tile([128, 2, D], _BF)
        qb_ap = qb[:, :]
        nc.sync.dma_start(out=qbc, in_=bass.AP(
            tensor=qb_ap.tensor, offset=qb_ap.offset,
            ap=[[0, 128]] + list(qb_ap.ap)))
        wa_sb = consts.tile([128, NCH, D], _BF)
        nc.sync.dma_start(out=wa_sb, in_=wa[:, :, :])
        wm_sb = consts.tile([128, NCH, D], _BF)
        nc.sync.dma_start(out=wm_sb, in_=wm[:, :, :])

        _I32 = mybir.dt.int32

        def rsqrt_dve(src_ap, w, tag):
            """y ~= rsqrt(src/D + eps) on DVE (Quake seed + 2 Newton iters)."""
            ms = stats.tile([128, w], _F32, tag=tag + "ms")
            nc.vector.tensor_scalar(out=ms, in0=src_ap, scalar1=1.0 / D,
                                    scalar2=EPS, op0=OP.mult, op1=OP.add)
            i32 = stats.tile([128, w], _I32, tag=tag + "i")
            nc.vector.tensor_scalar(out=i32, in0=ms.bitcast(_I32), scalar1=1,
                                    scalar2=-1, op0=OP.logical_shift_right,
                                    op1=OP.bitwise_xor)
            y0i = stats.tile([128, w], _I32, tag=tag + "y0")
            nc.vector.tensor_scalar(out=y0i, in0=i32, scalar1=1597463008,
                                    scalar2=None, op0=OP.add)
            ycur = y0i.bitcast(_F32)
            t1 = stats.tile([128, w], _F32, tag=tag + "t")
            for it in range(1):
                ynext = stats.tile([128, w], _F32, tag=tag + f"y{it}")
                nc.vector.tensor_mul(out=t1, in0=ycur, in1=ycur)
                nc.vector.tensor_mul(out=t1, in0=t1, in1=ms)
                nc.vector.tensor_scalar(out=t1, in0=t1, scalar1=-0.5,
                                        scalar2=1.5, op0=OP.mult, op1=OP.add)
                nc.vector.tensor_mul(out=ynext, in0=ycur, in1=t1)
                ycur = ynext
            return ycur

        state = {}

        def emit_load_stats(tt):
            v = vin.tile([128, NB, D], _BF, tag="v")
            for part in range(3):
                ns = slice(3 * part, 3 * part + 3)
                nc.sync.dma_start(out=v[:, ns, :], in_=vn[tt][:, ns, :])
            ssq = stats.tile([128, NB], _F32, tag="ssq")
            dots = stats.tile([128, 2, NB], _F32, tag="dots")

            # per-block reductions: ssq on ACT, dots split DVE/ACT
            for n in range(NB):
                ja = work.tile([128, D], _BF, tag=f"ja{n % 2}")
                nc.scalar.activation(out=ja, in_=v[:, n, :], func=AF.Square,
                                     accum_out=ssq[:, n:n + 1])
            for qi in range(2):
                for n in range(NB):
                    if (qi * NB + n) % 4 < 3:  # 14 on DVE-stt, 4 via ACT
                        jv = work.tile([128, D], _BF, tag=f"jv{n % 2}")
                        nc.vector.scalar_tensor_tensor(
                            out=jv, in0=v[:, n, :], scalar=1.0,
                            in1=qbc[:, qi, :], op0=OP.mult, op1=OP.mult,
                            accum_out=dots[:, qi, n:n + 1])
                    else:
                        pr = work.tile([128, D], _BF, tag=f"pr{n % 2}")
                        nc.vector.tensor_mul(out=pr, in0=v[:, n, :],
                                             in1=qbc[:, qi, :])
                        jb = work.tile([128, D], _BF, tag=f"jb{n % 2}")
                        nc.scalar.activation(out=jb, in_=pr, func=AF.Copy,
                                             accum_out=dots[:, qi, n:n + 1])
            state[tt] = dict(v=v, ssq=ssq, dots=dots, pcur=v[:, 8, :])

        def emit_softmax_diag(tt, phase):
            st = state[tt]
            ssq, dots = st["ssq"], st["dots"]
            # w_n = exp(dot_n * rsqrt(ssq_n/D + eps) / 32)
            rinv = rsqrt_dve(ssq[:, :], NB, f"r{phase}")
            lg = stats.tile([128, NB], _F32, tag=f"lg{phase}")
            nc.vector.tensor_mul(out=lg, in0=dots[:, phase, :], in1=rinv)
            ex = stats.tile([128, NB], _BF, tag=f"ex{phase}")
            nc.scalar.activation(out=ex, in_=lg, func=AF.Exp,
                                 scale=INV_SCALE)
            # diag(w_n) for all 9 blocks in one broadcast multiply
            diag = work.tile([128, NB, 128], _BF, tag=f"dg{phase}")
            ex_ap = ex[:, :]
            ex_bc = bass.AP(tensor=ex_ap.tensor, offset=ex_ap.offset,
                            ap=list(ex_ap.ap) + [[0, 128]])
            nc.vector.tensor_mul(out=diag, in0=ident9, in1=ex_bc)
            st[f"diag{phase}"] = diag

        def emit_h(tt, phase):
            st = state[tt]
            v, pcur, diag = st["v"], st["pcur"], st[f"diag{phase}"]
            h_ps = pbig.tile([128, D], _F32, tag="h")
            for half in range(2):
                hs = slice(512 * half, 512 * half + 512)
                for n in range(NB):
                    rhs = v[:, n, hs] if n < 8 else pcur[:, hs]
                    nc.tensor.matmul(h_ps[:, hs], lhsT=diag[:, n, :],
                                     rhs=rhs, start=(n == 0), stop=(n == 8))
            st[f"h{phase}"] = h_ps

        def emit_hn(tt, phase):
            st = state[tt]
            h_ps = st[f"h{phase}"]
            # cast h to bf16 unscaled; rmsnorm scale folds into the
            # post-GEMM fused op (GEMM is linear in h)
            ssqh = stats.tile([128, 1], _F32, tag=f"sh{phase}")
            jh = work.tile([128, D], _BF, tag="jh")
            nc.scalar.activation(out=jh, in_=h_ps, func=AF.Square,
                                 accum_out=ssqh)
            rih = rsqrt_dve(ssqh[:, :], 1, f"z{phase}")
            hn = work.tile([128, D], _BF, tag=f"hn{phase}")
            nc.scalar.activation(out=hn, in_=h_ps, func=AF.Copy)
            hnT = work.tile([128, NCH, 128], _BF, tag=f"ht{phase}")
            nc.sync.dma_start_transpose(hnT, hn)
            st[f"rih{phase}"] = rih
            st[f"hnT{phase}"] = hnT

        def emit_gemm(tt, phase):
            st = state[tt]
            hnT = st[f"hnT{phase}"]
            w_sb = wa_sb if phase == 0 else wm_sb
            g_ps = pbig.tile([128, D], _F32, tag="g")
            for half in range(2):
                hs = slice(512 * half, 512 * half + 512)
                for c in range(NCH):
                    nc.tensor.matmul(g_ps[:, hs], lhsT=hnT[:, c, :],
                                     rhs=w_sb[:, c, hs],
                                     start=(c == 0), stop=(c == NCH - 1))
            st[f"g{phase}"] = g_ps

        def emit_pout(tt, phase):
            st = state[tt]
            g_ps, rih, pcur = st[f"g{phase}"], st[f"rih{phase}"], st["pcur"]
            ssq, dots = st["ssq"], st["dots"]
            if phase == 0:
                # p1 in bf16 directly; it is the phase-2 residual base
                p1 = work.tile([128, D], _BF, tag="p1")
                nc.vector.scalar_tensor_tensor(
                    out=p1, in0=g_ps, scalar=rih[:, :], in1=pcur,
                    op0=OP.mult, op1=OP.add)
                # refresh block-8 stats for phase 2
                ja = work.tile([128, D], _BF, tag="ja0")
                nc.scalar.activation(out=ja, in_=p1, func=AF.Square,
                                     accum_out=ssq[:, 8:9])
                jv = work.tile([128, D], _BF, tag="jv0")
                nc.vector.scalar_tensor_tensor(
                    out=jv, in0=p1, scalar=1.0, in1=qbc[:, 1, :],
                    op0=OP.mult, op1=OP.mult,
                    accum_out=dots[:, 1, 8:9])
                st["pcur"] = p1
            else:
                pout = work.tile([128, D], _F32, tag="po1")
                nc.vector.scalar_tensor_tensor(
                    out=pout, in0=g_ps, scalar=rih[:, :], in1=pcur,
                    op0=OP.mult, op1=OP.add)
                nc.sync.dma_start(out=out[tt], in_=pout)
                del state[tt]

        # software pipeline over tiles: stats(i) | phase0(i-1) | phase1(i-2)
        # with per-engine interleaving of the two in-flight phases
        for i in range(NT + 2):
            a = i - 1   # tile in phase 0
            b = i - 2   # tile in phase 1
            if 0 <= a < NT:
                emit_softmax_diag(a, 0)
            if 0 <= b < NT:
                emit_softmax_diag(b, 1)
            if 0 <= a < NT:
                emit_h(a, 0)
            if 0 <= b < NT:
                emit_h(b, 1)
            if i < NT:
                emit_load_stats(i)
            if 0 <= a < NT:
                emit_hn(a, 0)
            if 0 <= b < NT:
                emit_hn(b, 1)
            if 0 <= a < NT:
                emit_gemm(a, 0)
            if 0 <= b < NT:
                emit_gemm(b, 1)
            if 0 <= a < NT:
                emit_pout(a, 0)
            if 0 <= b < NT:
                emit_pout(b, 1)

    nc.compile()
    return nc


def _get_nc():
    if "nc" not in _CACHE:
        _CACHE["nc"] = build_nc()
    return _CACHE["nc"]


def _prepare_in_maps(completed_blocks, partial_block, attn_norm_w, attn_w,
                     mlp_norm_w, mlp_w, attn_res_query, attn_res_norm_w,
                     mlp_res_query, mlp_res_norm_w):
    V = np.ascontiguousarray(
        np.asarray(completed_blocks, np.float32)).reshape(N_BLK, TOK, D)
    P = np.ascontiguousarray(
        np.asarray(partial_block, np.float32)).reshape(TOK, D)
    qwa = np.asarray(attn_res_query, np.float32) * np.asarray(attn_res_norm_w, np.float32)
    qwm = np.asarray(mlp_res_query, np.float32) * np.asarray(mlp_res_norm_w, np.float32)
    WaT = (np.asarray(attn_w, np.float32) * np.asarray(attn_norm_w, np.float32)[None, :]).T
    WmT = (np.asarray(mlp_w, np.float32) * np.asarray(mlp_norm_w, np.float32)[None, :]).T

    qb_host = np.ascontiguousarray(
        np.stack([qwa, qwm], axis=0).astype(bf16))            # [2, D]
    wa_host = np.ascontiguousarray(
        WaT.astype(bf16).reshape(NCH, 128, D).transpose(1, 0, 2))
    wm_host = np.ascontiguousarray(
        WmT.astype(bf16).reshape(NCH, 128, D).transpose(1, 0, 2))

    in_maps = []
    for c in range(NCORES):
        sl = slice(c * TPC, (c + 1) * TPC)
        Vc = V[:, sl, :].astype(bf16)                          # [8, 1024, 1024]
        Pc = P[sl].astype(bf16)                                # [1024, 1024]
        vn_host = np.empty((NT, 128, NB, D), dtype=bf16)
        vn_host[:, :, :8, :] = Vc.reshape(N_BLK, NT, 128, D).transpose(1, 2, 0, 3)
        vn_host[:, :, 8, :] = Pc.reshape(NT, 128, D)
        in_maps.append(dict(vn=vn_host, qb=qb_host, wa=wa_host, wm=wm_host))
    return in_maps


def _run(in_maps, **kw):
    nc = _get_nc()
    return run_bass_kernel_spmd(nc, in_maps, core_ids=list(range(NCORES)), **kw)


def kernel(completed_blocks, partial_block, attn_norm_w, attn_w, mlp_norm_w,
           mlp_w, attn_res_query, attn_res_norm_w, mlp_res_query,
           mlp_res_norm_w, layer_in_block=None, **_unused):
    in_maps = _prepare_in_maps(completed_blocks, partial_block, attn_norm_w,
                               attn_w, mlp_norm_w, mlp_w, attn_res_query,
                               attn_res_norm_w, mlp_res_query, mlp_res_norm_w)
    res = _run(in_maps)
    outs = [np.asarray(r["out"], np.float32).reshape(TPC, D) for r in res.results]
    return np.concatenate(outs, axis=0).reshape(B, T, D)
